# revision 6
# baseline (speedup 1.0000x reference)
"""Trainium2 Bass kernel for nn_MicroAdder_16501264351743.

2-layer dense transformer, B=4 T=1024 D=1024, split-subspace attention with
tied QK, GQA 16/4 heads, q-phase rotation, ALiBi with slope +log(10), FFN 4096.

Key structural facts exploited (verified against the fp32 reference):
  * ALiBi bias is slope*(i-j) with slope=+log(10)=2.3026 — softmax mass
    concentrates on the FIRST keys of the sequence.  In fp32 the reference's
    own softmax gives exactly-zero weight to every key j>=64 (max nonzero key
    index is 44).  We compute attention over the first NKEY=64 keys only,
    which is exact at fp32 granularity.
  * softmax(qk + slope*(i-j)) == softmax(qk - slope*j) (row-constant shift),
    and logits are small (|qk|<20), so exp() without max-subtraction is safe.
  * The q-phase rotation, qk scale, and all rmsnorm weights fold into the
    projection weights on the host.

Sharding: 8 cores, core pair (2b, 2b+1) per batch b; no collectives.  K/V
come only from tokens [0,64), so each core carries a private copy of those
64 key tokens at slots [0:64) BEFORE its 512 output tokens (core 2b owns
outputs [0,512), core 2b+1 owns [512,1024)).  Keys-at-front means chunk A
(cols 0..) contains the keys, so the next layer's K/V inputs are finished by
FFN2's FIRST chunk and the next layer's attention front-end can interleave
into FFN2's second chunk (see schedule below).  Layer 0 evolves all 576
slots (the keys' residual stream feeds layer 1's K/V); layer 1 and the head
run on slots [64:576).  Causal masks are per-core input data (even cores
causal, odd cores all-ones) so the program stays SPMD-uniform.

Layout: activations persist TRANSPOSED in SBUF: [128 partitions, slab, token]
with feature = slab*128 + partition.  Every matmul is then
out[feat', tok] = W[feat, feat']^T @ act[feat, tok] — no transposes anywhere.
rmsnorm's partition-dim reduction is an all-ones matmul; 1/sqrt comes from
scalar Sqrt + the fast custom-DVE reciprocal.

Softmax normalization runs almost entirely on the PE: scores (block-diag K
per head pair) -> exp (+alibi bias as per-partition bias, ScalarE) ->
per-head denominators accumulated into ONE [16,tok] PSUM via per-pair masked
ones matmuls -> one copy + fast reciprocal -> reciprocal broadcast to 128
partitions with a tiny per-pair selector matmul, applied to the unnormalized
AV output with one DVE mul per pair.

Pipelined schedule (the PE HAM halves the clock during sparse phases, so
every latency chain must hide under independent matmul work):
  S1: dn(A), qT(B), av(A), scores(B), outproj(A), norm2(A), dn(B), av(B)
  S2: outproj(B), norm2(B), FFN1 m-loop (both chunks; first 4 m's chunk-B
      groups deferred until norm2(B)'s chain is covered)
  S3: FFN2(A) + woven h_next sq + finish(A)   [A contains the keys]
  S4: FFN2(B) interleaved with the NEXT layer's qT(A'), make_kv', scores(A')
      (or the head's chunk-A groups on the last layer) + finish(B)
qT/head PSUM->SBUF copies live on DVE so ScalarE is free for exp chains;
the exp chains always land before the next dn because gelus/exps alternate
at S2/S4 boundaries.  FFN2 weights are streamed twice (S3/S4 chunk-outer
order) — +8MB DMA, well under the DMA roofline.  Startup: first 128-col x
piece + ssq `ones` land first; first-layer norm runs in 128-col pieces with
sq on DVE (gpsimd's serial 0.6us/slab would gate the chain).
Keep gpsimd lightly loaded: heavy co-activity down-clocks the PE ~20%.
"""

import numpy as np
import ml_dtypes

import concourse.bass as bass
import concourse.mybir as mybir
import concourse.tile as tile
from concourse import bacc
from concourse.bass_utils import run_bass_kernel_spmd

F32 = mybir.dt.float32
BF16 = mybir.dt.bfloat16
AF = mybir.ActivationFunctionType
ALU = mybir.AluOpType
BF = ml_dtypes.bfloat16

B, T, L = 4, 1024, 2
D, TOKD, POSD = 1024, 512, 512
H, HD, KVH, FFN = 16, 64, 4, 4096
INNER, KVI, REP = 1024, 256, 4
EPS = 1e-5

NKEY = 64           # keys that can carry softmax mass (last nonzero: 44)
NTOK = 576          # layer-0 slots per core (64 keys + 512 outputs)
NOUT = 512          # output slots per core
KOFF = 0            # key slots [0, NKEY)
YOFF = 64           # output slots [YOFF, YOFF+NOUT)
CHUNKS0 = [(0, 256), (256, 320)]     # layer 0: all 576 slots
# layer-1 chunk A must sit inside layer-0's chunk-A norm (cols 0:256), since
# the next layer's Q matmuls are emitted inside layer-0's FFN2(B) phase
CHUNKS1 = [(64, 192), (256, 320)]    # layer 1 / head: slots 64:576
CHMAX = 320
NCORES = 8


# ----------------------------------------------------------------------------
# host-side weight preparation
# ----------------------------------------------------------------------------

def _prep_weights(inputs):
    """Fold norms/rotation/scale into weights; emit SBUF-image numpy arrays."""
    qW = np.asarray(inputs["qW"], np.float32)
    vW = np.asarray(inputs["vW"], np.float32)
    oW = np.asarray(inputs["oW"], np.float32)
    ln1 = np.asarray(inputs["ln1_w"], np.float32)
    ln2 = np.asarray(inputs["ln2_w"], np.float32)
    lnf = np.asarray(inputs["lnf_w"], np.float32)
    fc1 = np.asarray(inputs["fc1_W"], np.float32)
    fc2 = np.asarray(inputs["fc2_W"], np.float32)
    fc1_b = np.asarray(inputs["fc1_b"], np.float32)
    fc2_b = np.asarray(inputs["fc2_b"], np.float32)
    headW = np.asarray(inputs["head_W"], np.float32)
    ang = np.asarray(inputs["q_phase_angle"], np.float32)
    slopes = np.exp(np.asarray(inputs["alibi_log_slopes"], np.float32))

    out = {}
    qW_l, kW_l, vW_l, oW_l, f1_l, f2_l = [], [], [], [], [], []
    for l in range(L):
        ln1_tok, ln1_pos = ln1[l, :TOKD], ln1[l, TOKD:]
        qW_e = qW[l] * ln1_pos[:, None]          # [512, 1024] folded ln1
        # K uses the UNrotated, UNscaled first KVI columns
        kW_e = qW_e[:, :KVI].copy()              # [512, 256]
        # rotate q per head then fold 1/sqrt(HD)
        qr = qW_e.reshape(POSD, H, HD // 2, 2)
        c = np.cos(ang[l])[None, :, None]
        s = np.sin(ang[l])[None, :, None]
        e, o = qr[..., 0].copy(), qr[..., 1].copy()
        qr[..., 0] = c * e - s * o
        qr[..., 1] = s * e + c * o
        qW_e = qr.reshape(POSD, INNER) * np.float32(1.0 / np.sqrt(HD))
        vW_e = vW[l] * ln1_tok[:, None]          # [512, 256]
        f1_e = fc1[l] * ln2[l][:, None]          # [1024, 4096]

        # SBUF images (lhsT layout: [partition=k%128, kslab, mcols])
        qW_l.append(qW_e.reshape(4, 128, INNER).transpose(1, 0, 2))
        # kW duplicated per kv-head so each q-head can matmul at its own
        # partition base: [128, ks, g, 128] with cols 0:64==64:128==head g
        kw = np.empty((POSD, KVH, 128), np.float32)
        for g in range(KVH):
            blk = kW_e[:, g * HD:(g + 1) * HD]
            kw[:, g, :HD] = blk
            kw[:, g, HD:] = blk
        kW_l.append(kw.reshape(4, 128, KVH, 128).transpose(1, 0, 2, 3))
        vW_l.append(vW_e.reshape(4, 128, KVI).transpose(1, 0, 2))
        oW_l.append(oW[l].reshape(8, 128, D).transpose(1, 0, 2))
        f1_l.append(f1_e.reshape(8, 128, 32, 128).transpose(2, 1, 0, 3))
        f2_l.append(fc2[l].reshape(32, 128, 8, 128).transpose(2, 1, 0, 3))

    out["qW"] = np.ascontiguousarray(np.stack(qW_l)).astype(BF)
    out["kW"] = np.ascontiguousarray(np.stack(kW_l)).astype(BF)
    out["vW"] = np.ascontiguousarray(np.stack(vW_l)).astype(BF)
    out["oW"] = np.ascontiguousarray(np.stack(oW_l)).astype(BF)
    out["f1"] = np.ascontiguousarray(np.stack(f1_l)).astype(BF)
    out["f2"] = np.ascontiguousarray(np.stack(f2_l)).astype(BF)
    hW_e = headW * lnf[:, None]
    out["hW"] = np.ascontiguousarray(
        hW_e.reshape(8, 128, TOKD).transpose(1, 0, 2)).astype(BF)

    # exp bias: -slope * key_index, per partition (keys of the head pair)
    kb = np.empty((128, L, H // 2), np.float32)
    jj = np.arange(64, dtype=np.float32)
    for l in range(L):
        for pr in range(H // 2):
            kb[0:64, l, pr] = -slopes[l, 2 * pr] * jj
            kb[64:128, l, pr] = -slopes[l, 2 * pr + 1] * jj
    out["kb"] = kb
    fb1 = np.zeros((128, L, 32), np.float32)
    fb2 = np.zeros((128, L, 8), np.float32)
    for l in range(L):
        fb1[:, l, :] = fc1_b[l].reshape(32, 128).T
        fb2[:, l, :] = fc2_b[l].reshape(8, 128).T
    # f32 consts packed into one DMA: kb | fb1 | fb2 | eps
    cpf = np.concatenate([kb.reshape(128, 16), fb1.reshape(128, 64),
                          fb2.reshape(128, 16),
                          np.full((128, 1), EPS, np.float32)], axis=1)
    out["cpf"] = np.ascontiguousarray(cpf)
    # per-pair denominator reduction lhsT: [128, pr, 16]; pair pr sums its
    # two heads' key rows into output partitions 2pr (head A) / 2pr+1 (head B)
    dn16 = np.zeros((128, 8, 16), np.float32)
    for pr in range(8):
        dn16[0:64, pr, 2 * pr] = 1.0
        dn16[64:128, pr, 2 * pr + 1] = 1.0
    # bf16 consts packed (per-core cmA appended in _make_in_maps):
    # ones | dn16 | cmA
    out["cpb_shared"] = np.concatenate(
        [np.ones((128, 128), BF), dn16.reshape(128, 128).astype(BF)], axis=1)
    # reciprocal broadcast lhsT per pair: [16, pr, 128]; output row c picks
    # r16 row 2pr (c<64) or 2pr+1 (c>=64)
    selb = np.zeros((16, 8, 128), np.float32)
    for pr in range(8):
        selb[2 * pr, pr, 0:64] = 1.0
        selb[2 * pr + 1, pr, 64:128] = 1.0
    out["selb"] = selb.astype(BF)
    return out


def _core_token_slices(core):
    """Global token rows for this core's 576-row local tensor:
    the 64 key tokens FIRST, then 512 output tokens."""
    b = core // 2
    if core % 2 == 0:
        return b, [(0, 64), (0, 512)]
    return b, [(0, 64), (512, 1024)]


def _make_xt(x, core):
    b, sls = _core_token_slices(core)
    rows = np.concatenate([x[b, a:c] for a, c in sls], axis=0)  # [576, 1024]
    assert rows.shape == (NTOK, D)
    xt = rows.T.reshape(8, 128, NTOK).transpose(1, 0, 2)        # [128, 8, 576]
    return np.ascontiguousarray(xt, dtype=np.float32)


def _make_cmA(core):
    """Mask for chunk-A's first 128 cols.  Cols 0:64 are the key tokens as
    queries (uniform causal).  Cols 64:128 are output tokens 0:64 (even
    cores: causal) or 512:576 (odd cores: all keys visible)."""
    j = np.arange(NKEY)
    causal = (j[:, None] <= j[None, :]).astype(BF)   # keep key j <= query i
    blk2 = causal if core % 2 == 0 else np.ones((NKEY, NKEY), BF)
    half = np.concatenate([causal, blk2], axis=1)    # [64, 128]
    return np.ascontiguousarray(np.concatenate([half, half], axis=0))


def _make_cpb(w, core):
    return np.ascontiguousarray(
        np.concatenate([w["cpb_shared"], _make_cmA(core)], axis=1))


# ----------------------------------------------------------------------------
# device kernel
# ----------------------------------------------------------------------------

_NC_CACHE = {}


def _build_nc():
    if "nc" in _NC_CACHE:
        return _NC_CACHE["nc"]
    nc = bacc.Bacc("TRN2", target_bir_lowering=False, debug=False,
                   num_devices=NCORES)

    xT_d = nc.dram_tensor("xT", [128, 8, NTOK], F32, kind="ExternalInput")
    qW_d = nc.dram_tensor("qW", [L, 128, 4, INNER], BF16, kind="ExternalInput")
    kW_d = nc.dram_tensor("kW", [L, 128, 4, KVH, 128], BF16, kind="ExternalInput")
    vW_d = nc.dram_tensor("vW", [L, 128, 4, KVI], BF16, kind="ExternalInput")
    oW_d = nc.dram_tensor("oW", [L, 128, 8, D], BF16, kind="ExternalInput")
    f1_d = nc.dram_tensor("f1", [L, 32, 128, 8, 128], BF16, kind="ExternalInput")
    f2_d = nc.dram_tensor("f2", [L, 8, 128, 32, 128], BF16, kind="ExternalInput")
    hW_d = nc.dram_tensor("hW", [128, 8, TOKD], BF16, kind="ExternalInput")
    cpf_d = nc.dram_tensor("cpf", [128, 97], F32, kind="ExternalInput")
    cpb_d = nc.dram_tensor("cpb", [128, 384], BF16, kind="ExternalInput")
    selb_d = nc.dram_tensor("selb", [16, 8, 128], BF16, kind="ExternalInput")
    y_d = nc.dram_tensor("y", [128, 4, NOUT], F32, kind="ExternalOutput")

    with tile.TileContext(nc) as tc:
        with (
            tc.tile_pool(name="const", bufs=1) as const,
            tc.tile_pool(name="persist", bufs=1) as persist,
            tc.tile_pool(name="act", bufs=1) as act,
            tc.tile_pool(name="wpool", bufs=1) as wpool,
            tc.tile_pool(name="wstream", bufs=4) as wstream,
            tc.tile_pool(name="small", bufs=2) as small,
            tc.tile_pool(name="attn", bufs=1) as attn,
            tc.tile_pool(name="ps", bufs=8, space="PSUM") as ps,
        ):
            # Startup DMAs, ordered for the first-norm critical path: the
            # first 128-col x piece (contains the keys) and the ssq `ones`
            # land first, then qW/kW/vW for Q + make_kv, then the rest.
            xT = persist.tile([128, 8, NTOK], F32)
            cpf_t = const.tile([128, 97], F32)
            cpb_t = const.tile([128, 384], BF16)
            selb_t = const.tile([16, 8, 128], BF16)
            nc.sync.dma_start(xT[:, :, 0:128], xT_d.ap()[:, :, 0:128])
            nc.sync.dma_start(cpb_t[:], cpb_d.ap())
            nc.sync.dma_start(cpf_t[:], cpf_d.ap())
            nc.sync.dma_start(xT[:, :, 128:256], xT_d.ap()[:, :, 128:256])
            nc.sync.dma_start(selb_t[:], selb_d.ap())
            kb_t = cpf_t[:, 0:16].rearrange("p (l h) -> p l h", l=L)
            fb1_t = cpf_t[:, 16:80].rearrange("p (l m) -> p l m", l=L)
            fb2_t = cpf_t[:, 80:96].rearrange("p (l m) -> p l m", l=L)
            eps_t = cpf_t[:, 96:97]
            ones_t = cpb_t[:, 0:128]
            dn16_t = cpb_t[:, 128:256].rearrange("p (r c) -> p r c", r=8)
            cmA_t = cpb_t[:, 256:384]

            def load_weights(l):
                qW_t = wpool.tile([128, 4, INNER], BF16, tag="qw", name="qW_t")
                nc.sync.dma_start(qW_t[:], qW_d.ap()[l])
                kW_t = wpool.tile([128, 4, KVH, 128], BF16, tag="kw",
                                  name="kW_t")
                nc.sync.dma_start(kW_t[:], kW_d.ap()[l])
                vW_t = wpool.tile([128, 4, KVI], BF16, tag="vw", name="vW_t")
                nc.sync.dma_start(vW_t[:], vW_d.ap()[l])
                if l == 0:
                    nc.sync.dma_start(xT[:, :, 256:576],
                                      xT_d.ap()[:, :, 256:576])
                oW_t = wpool.tile([128, 8, D], BF16, tag="ow", name="oW_t")
                nc.sync.dma_start(oW_t[:], oW_d.ap()[l])
                return qW_t, kW_t, vW_t, oW_t

            w0 = load_weights(0)

            # block-diagonal K^T and V per kv-group: [[M_g, 0], [0, M_g]].
            kT2 = persist.tile([128, KVH, 128], BF16)
            v2 = persist.tile([128, KVH, 128], BF16)
            nc.vector.memset(kT2[:], 0.0)
            nc.vector.memset(v2[:], 0.0)

            def norm_sq(sq_t, c0, cn, s, eng=None):
                eng = eng or nc.gpsimd
                eng.tensor_mul(sq_t[:, s, c0:c0 + cn],
                               xT[:, s, c0:c0 + cn],
                               xT[:, s, c0:c0 + cn])

            def norm_finish(out_bf, sq_t, c0, cn):
                """out_bf[:, :, c0:c0+cn] = rmsnorm(xT) (ln weight folded).
                pos-half slabs (4..7) first so Q matmuls can start early."""
                ssq = ps.tile([128, 512], F32, tag="ps")
                for s in range(8):
                    nc.tensor.matmul(ssq[:, :cn], lhsT=ones_t[:],
                                     rhs=sq_t[:, s, c0:c0 + cn],
                                     start=(s == 0), stop=(s == 7))
                sr = small.tile([128, CHMAX], F32, tag="sr", bufs=4)
                nc.scalar.activation(sr[:, :cn], ssq[:, :cn],
                                     AF.Sqrt, bias=eps_t[:, 0:1], scale=1.0 / D)
                nc.vector.reciprocal_approx_fast(sr[:, :cn], sr[:, :cn])
                sr_b4 = sr[:, :cn].unsqueeze(1).broadcast_to([128, 4, cn])
                nc.vector.tensor_mul(out_bf[:, 4:8, c0:c0 + cn],
                                     xT[:, 4:8, c0:c0 + cn], sr_b4)
                nc.vector.tensor_mul(out_bf[:, 0:4, c0:c0 + cn],
                                     xT[:, 0:4, c0:c0 + cn], sr_b4)

            def norm_chunk(out_bf, sq_t, c0, cn, eng=None, split=False):
                for s in range(8):
                    e = eng
                    if split:  # pos-half on DVE (feeds Q first), rest gpsimd
                        e = nc.vector if s >= 4 else nc.gpsimd
                    norm_sq(sq_t, c0, cn, s, eng=e)
                norm_finish(out_bf, sq_t, c0, cn)

            hT0 = act.tile([128, 8, NTOK], BF16, tag="hT", name="hT0")
            sq1 = act.tile([128, 8, NTOK], BF16, tag="sq", name="sq1")
            # First two 128-col pieces of the layer-0 norm: sq on DVE (idle
            # at startup; gpsimd's serial 0.6us/slab would gate the chain).
            norm_chunk(hT0, sq1, 0, 128, eng=nc.vector)
            norm_chunk(hT0, sq1, 128, 128, eng=nc.vector)

            hW_t = const.tile([128, 8, TOKD], BF16)

            # ------------------------------------------------------------
            # per-layer helpers, parameterized by a small state dict
            # ------------------------------------------------------------

            def new_state(l, hT, wts):
                return {
                    "l": l, "hT": hT,
                    "qW": wts[0], "kW": wts[1], "vW": wts[2], "oW": wts[3],
                    "qT": act.tile([128, 8, NTOK], BF16, tag="qT",
                                   name=f"qT{l}"),
                    "oT": act.tile([128, 8, NTOK], BF16, tag="oT",
                                   name=f"oT{l}"),
                    "exps": {}, "r16": {},
                }

            def make_qT(st, c0, cn):
                for ms in range(8):
                    q_ps = ps.tile([128, 512], F32, tag="ps")
                    for s in range(4):
                        nc.tensor.matmul(
                            q_ps[:, :cn],
                            lhsT=st["qW"][:, s, ms * 128:(ms + 1) * 128],
                            rhs=st["hT"][:, 4 + s, c0:c0 + cn],
                            start=(s == 0), stop=(s == 3))
                    nc.vector.tensor_copy(st["qT"][:, ms, c0:c0 + cn],
                                          q_ps[:, :cn])

            def make_kv(st):
                hT = st["hT"]
                # V: keys (slots 0:64), replicated on both partition halves
                v_ps = ps.tile([128, 512], F32, tag="ps")
                for part in (0, 64):
                    for s in range(4):
                        nc.tensor.matmul(v_ps[part:part + 64, :KVI],
                                         lhsT=hT[:, s, 0:NKEY],
                                         rhs=st["vW"][:, s, :],
                                         start=(s == 0), stop=(s == 3))
                for g in range(KVH):
                    nc.vector.tensor_copy(v2[0:64, g, 0:64],
                                          v_ps[0:64, g * HD:(g + 1) * HD])
                    nc.vector.tensor_copy(v2[64:128, g, 64:128],
                                          v_ps[64:128, g * HD:(g + 1) * HD])
                # K^T diagonal blocks
                for g in range(KVH):
                    k_ps = ps.tile([128, 512], F32, tag="ps")
                    for s in range(4):
                        nc.tensor.matmul(k_ps[:, :NKEY],
                                         lhsT=st["kW"][:, s, g, :],
                                         rhs=hT[:, 4 + s, 0:NKEY],
                                         start=(s == 0), stop=(s == 3))
                    nc.vector.tensor_copy(kT2[0:64, g, 0:64],
                                          k_ps[0:64, :NKEY])
                    nc.vector.tensor_copy(kT2[64:128, g, 64:128],
                                          k_ps[64:128, :NKEY])

            def attn_scores(st, ch_idx, c0, cn):
                """exp(scores+alibi) for all pairs (scores matmul + exp on
                ScalarE + causal-mask muls on gpsimd)."""
                l = st["l"]
                exps = []
                for g in range(KVH):
                    for pr in (2 * g, 2 * g + 1):
                        s_ps = ps.tile([128, 512], F32, tag="ps")
                        nc.tensor.matmul(s_ps[:, :cn], lhsT=kT2[:, g, :],
                                         rhs=st["qT"][:, pr, c0:c0 + cn],
                                         start=True, stop=True)
                        e1 = attn.tile([128, CHMAX], BF16, tag="e1",
                                       bufs=16, name="e1")
                        nc.scalar.activation(e1[:, :cn], s_ps[:, :cn],
                                             AF.Exp,
                                             bias=kb_t[:, l, pr:pr + 1])
                        if ch_idx == 0:
                            if l == 0:
                                # cols 0:64 key-queries (uniform causal),
                                # cols 64:128 per-core
                                nc.gpsimd.tensor_mul(e1[:, 0:128],
                                                     e1[:, 0:128], cmA_t[:])
                            else:
                                # chunk starts at slot 64: only cols 0:64
                                # (output tokens 0:64 / 512:576) need a mask
                                nc.gpsimd.tensor_mul(e1[:, 0:NKEY],
                                                     e1[:, 0:NKEY],
                                                     cmA_t[:, 64:128])
                        exps.append(e1)
                st["exps"][ch_idx] = exps

            def attn_dn(st, ch_idx, c0, cn):
                """Per-head denominators -> one [16,cn] PSUM -> 1/d -> r16.
                Emitted after independent matmul work so the PE isn't parked
                behind the exp chain."""
                exps = st["exps"][ch_idx]
                dn_ps = ps.tile([128, 512], F32, tag="ps")
                for pr in range(8):
                    nc.tensor.matmul(dn_ps[0:16, :cn],
                                     lhsT=dn16_t[:, pr, :],
                                     rhs=exps[pr][:, :cn],
                                     start=(pr == 0), stop=(pr == 7))
                dnsb = attn.tile([16, CHMAX], F32, tag="dnsb", bufs=2)
                nc.vector.tensor_copy(dnsb[:, :cn], dn_ps[0:16, :cn])
                nc.vector.reciprocal_approx_fast(dnsb[:, :cn], dnsb[:, :cn])
                r16 = attn.tile([16, CHMAX], BF16, tag="r16", bufs=2)
                nc.vector.tensor_copy(r16[:, :cn], dnsb[:, :cn])
                st["r16"][ch_idx] = r16

            def attn_av(st, ch_idx, c0, cn):
                """AV (unnormalized), broadcast 1/denom via rank-2 matmul,
                normalize into oT with one DVE mul per pair."""
                r16 = st["r16"][ch_idx]
                exps = st["exps"][ch_idx]
                for g in range(KVH):
                    for pr in (2 * g, 2 * g + 1):
                        av_ps = ps.tile([128, 512], F32, tag="ps")
                        nc.tensor.matmul(av_ps[:, :cn], lhsT=v2[:, g, :],
                                         rhs=exps[pr][:, :cn],
                                         start=True, stop=True)
                        rb_ps = ps.tile([128, 512], F32, tag="ps")
                        nc.tensor.matmul(rb_ps[:, :cn],
                                         lhsT=selb_t[:, pr, :],
                                         rhs=r16[0:16, :cn],
                                         start=True, stop=True)
                        rb_sb = attn.tile([128, CHMAX], BF16, tag="rb",
                                          bufs=3, name="rb_sb")
                        nc.vector.tensor_copy(rb_sb[:, :cn], rb_ps[:, :cn])
                        nc.vector.tensor_mul(st["oT"][:, pr, c0:c0 + cn],
                                             av_ps[:, :cn], rb_sb[:, :cn])

            def outproj(st, c0, cn):
                for ms in range(8):
                    o_ps = ps.tile([128, 512], F32, tag="ps")
                    for ks in range(8):
                        nc.tensor.matmul(
                            o_ps[:, :cn],
                            lhsT=st["oW"][:, ks, ms * 128:(ms + 1) * 128],
                            rhs=st["oT"][:, ks, c0:c0 + cn],
                            start=(ks == 0), stop=(ks == 7))
                    nc.vector.tensor_add(xT[:, ms, c0:c0 + cn],
                                         o_ps[:, :cn],
                                         xT[:, ms, c0:c0 + cn])

            def head_group(hf, m, c0, cn):
                yst = small.tile([128, CHMAX], F32, tag="yst", bufs=4)
                y_ps = ps.tile([128, 512], F32, tag="ps")
                # contract pos-half slabs first: the final norm finishes
                # them first, so the head can start earlier
                for ks in (4, 5, 6, 7, 0, 1, 2, 3):
                    nc.tensor.matmul(y_ps[:, :cn],
                                     lhsT=hW_t[:, ks, m * 128:(m + 1) * 128],
                                     rhs=hf[:, ks, c0:c0 + cn],
                                     start=(ks == 4), stop=(ks == 3))
                nc.vector.tensor_copy(yst[:, :cn], y_ps[:, :cn])
                nc.sync.dma_start(y_d.ap()[:, m, c0 - YOFF:c0 - YOFF + cn],
                                  yst[:, :cn])

            # ------------------------------------------------------------
            # layer 0 prologue
            # ------------------------------------------------------------
            st = new_state(0, hT0, w0)
            make_qT(st, 0, 128)
            make_kv(st)
            make_qT(st, 128, 128)
            attn_scores(st, 0, *CHUNKS0[0])
            # ln1 for chunk B: pos-half sq on DVE (feeds qT(B)), rest gpsimd
            norm_chunk(hT0, sq1, 256, 320, split=True)

            for l in range(L):
                A, Bc = (CHUNKS0 if l == 0 else CHUNKS1)
                hT = st["hT"]

                if l == L - 1:
                    nc.sync.dma_start(hW_t[:], hW_d.ap())

                # ---- S1: attention back-half, chains hidden under matmuls
                if l == 0:
                    make_qT(st, *Bc)      # ln1(B) lands before exps finish
                    attn_dn(st, 0, *A)
                else:
                    attn_dn(st, 0, *A)    # exps(A) ran during prev S4
                    make_qT(st, *Bc)
                attn_av(st, 0, *A)
                attn_scores(st, 1, *Bc)
                outproj(st, *A)
                h2 = act.tile([128, 8, NTOK], BF16, tag="hT2", name=f"h2_{l}")
                sq2 = act.tile([128, 8, NTOK], BF16, tag="sq", name=f"sq2_{l}")
                norm_chunk(h2, sq2, *A)   # chain hidden under dn/av/outproj B
                attn_dn(st, 1, *Bc)
                attn_av(st, 1, *Bc)

                # ---- S2: outproj(B) + FFN1 (both chunks per m)
                outproj(st, *Bc)
                norm_chunk(h2, sq2, *Bc)
                h_next = act.tile([128, 8, NTOK], BF16, tag="hT",
                                  name=f"h_next{l}")
                sq_next = act.tile([128, 8, NTOK], BF16, tag="sq",
                                   name=f"sq_next{l}")
                gT = act.tile([128, 32, NTOK], BF16, tag="gT", name=f"gT{l}")

                def f1_group(m, f1w, c0, cn):
                    f_ps = ps.tile([128, 512], F32, tag="ps")
                    for ks in range(8):
                        nc.tensor.matmul(f_ps[:, :cn], lhsT=f1w[:, ks, :],
                                         rhs=h2[:, ks, c0:c0 + cn],
                                         start=(ks == 0), stop=(ks == 7))
                    nc.scalar.activation(gT[:, m, c0:c0 + cn], f_ps[:, :cn],
                                         AF.Gelu, bias=fb1_t[:, l, m:m + 1])

                # first 4 m's run chunk-A only, deferring their chunk-B
                # groups until norm2(B)'s finish chain has completed
                pend_f1 = []
                for m in range(32):
                    f1w = wstream.tile([128, 8, 128], BF16, tag="f1w")
                    nc.sync.dma_start(f1w[:], f1_d.ap()[l, m])
                    f1_group(m, f1w, *A)
                    if m < 4:
                        pend_f1.append((m, f1w))
                    else:
                        f1_group(m, f1w, *Bc)
                    if m == 3:
                        for mm, fw in pend_f1:
                            f1_group(mm, fw, *Bc)
                if l + 1 < L:
                    w_next = load_weights(l + 1)

                def ffn2_group(ms, f2w_h, c0, cn):
                    f_ps = ps.tile([128, 512], F32, tag="ps")
                    for ks in range(32):
                        nc.tensor.matmul(f_ps[:, :cn],
                                         lhsT=f2w_h[ks // 16][:, ks % 16, :],
                                         rhs=gT[:, ks, c0:c0 + cn],
                                         start=(ks == 0), stop=(ks == 31))
                    nc.vector.scalar_tensor_tensor(
                        xT[:, ms, c0:c0 + cn], f_ps[:, :cn],
                        fb2_t[:, l, ms:ms + 1], xT[:, ms, c0:c0 + cn],
                        op0=ALU.add, op1=ALU.add)

                def load_f2w(ms):
                    f2w_a = wstream.tile([128, 16, 128], BF16, tag="f2w")
                    nc.sync.dma_start(f2w_a[:], f2_d.ap()[l, ms][:, 0:16, :])
                    f2w_b = wstream.tile([128, 16, 128], BF16, tag="f2w")
                    nc.sync.dma_start(f2w_b[:], f2_d.ap()[l, ms][:, 16:32, :])
                    return [f2w_a, f2w_b]

                # ---- S3: FFN2(A) + woven h_next sq + finish(A)
                # (A contains the keys -> next layer's K/V input is ready)
                for ms in range(8):
                    ffn2_group(ms, load_f2w(ms), *A)
                    norm_sq(sq_next, A[0], A[1], ms)
                norm_finish(h_next, sq_next, *A)

                # ---- S4: FFN2(B) interleaved with next layer's front-end
                # (or the head's chunk-A groups on the last layer)
                if l + 1 < L:
                    st_next = new_state(l + 1, h_next, w_next)
                nextA = CHUNKS1[0]
                for ms in range(8):
                    ffn2_group(ms, load_f2w(ms), *Bc)
                    norm_sq(sq_next, Bc[0], Bc[1], ms)
                    if l + 1 < L:
                        if ms == 1:
                            make_qT(st_next, *nextA)
                        elif ms == 3:
                            make_kv(st_next)
                        elif ms == 5:
                            attn_scores(st_next, 0, *nextA)
                    else:
                        if 2 <= ms <= 5:
                            head_group(h_next, ms - 2, *CHUNKS1[0])
                norm_finish(h_next, sq_next, *Bc)

                if l + 1 < L:
                    st = st_next

            # ---- tail: head chunk B
            for m in range(4):
                head_group(h_next, m, *CHUNKS1[1])

    nc.compile()
    _NC_CACHE["nc"] = nc
    return nc


# ----------------------------------------------------------------------------
# entry point
# ----------------------------------------------------------------------------

WKEYS = ("qW", "kW", "vW", "oW", "f1", "f2", "hW", "cpf", "selb")


def _make_in_maps(inputs):
    x = np.asarray(inputs["x"], np.float32)
    w = _prep_weights(inputs)
    in_maps = []
    for core in range(NCORES):
        m = {k: w[k] for k in WKEYS}
        m["xT"] = _make_xt(x, core)
        m["cpb"] = _make_cpb(w, core)
        in_maps.append(m)
    return in_maps


def kernel(**inputs) -> np.ndarray:
    nc = _build_nc()
    in_maps = _make_in_maps(inputs)

    res = run_bass_kernel_spmd(nc, in_maps, core_ids=list(range(NCORES)))
    out = np.empty((B, T, TOKD), np.float32)
    for core in range(NCORES):
        yb = np.asarray(res.results[core]["y"])          # [128, 4, 512]
        yl = yb.transpose(2, 1, 0).reshape(NOUT, TOKD)   # [512, 512]
        b = core // 2
        if core % 2 == 0:
            out[b, 0:512] = yl
        else:
            out[b, 512:1024] = yl
    return out


# revision 10
# speedup vs baseline: 1.0248x; 1.0248x over previous
"""Trainium2 Bass kernel for nn_MicroAdder_16501264351743.

2-layer dense transformer, B=4 T=1024 D=1024, split-subspace attention with
tied QK, GQA 16/4 heads, q-phase rotation, ALiBi with slope +log(10), FFN 4096.

Key structural facts exploited (verified against the fp32 reference):
  * ALiBi bias is slope*(i-j) with slope=+log(10)=2.3026 — softmax mass
    concentrates on the FIRST keys of the sequence.  In fp32 the reference's
    own softmax gives exactly-zero weight to every key j>=64 (max nonzero key
    index is 44).  We compute attention over the first NKEY=64 keys only,
    which is exact at fp32 granularity.
  * softmax(qk + slope*(i-j)) == softmax(qk - slope*j) (row-constant shift),
    and logits are small (|qk|<20), so exp() without max-subtraction is safe.
  * The q-phase rotation, qk scale, and all rmsnorm weights fold into the
    projection weights on the host.

Sharding: 8 cores, core pair (2b, 2b+1) per batch b; no collectives.  K/V
come only from tokens [0,64), so each core carries a private copy of those
64 key tokens at slots [0:64) BEFORE its 512 output tokens (core 2b owns
outputs [0,512), core 2b+1 owns [512,1024)).  Keys-at-front means chunk A
(cols 0..) contains the keys, so the next layer's K/V inputs are finished by
FFN2's FIRST chunk and the next layer's attention front-end can interleave
into FFN2's second chunk (see schedule below).  Layer 0 evolves all 576
slots (the keys' residual stream feeds layer 1's K/V); layer 1 and the head
run on slots [64:576).  Causal masks are per-core input data (even cores
causal, odd cores all-ones) so the program stays SPMD-uniform.

Layout: activations persist TRANSPOSED in SBUF: [128 partitions, slab, token]
with feature = slab*128 + partition.  Every matmul is then
out[feat', tok] = W[feat, feat']^T @ act[feat, tok] — no transposes anywhere.
rmsnorm's partition-dim reduction is an all-ones matmul; 1/sqrt comes from
scalar Sqrt + the fast custom-DVE reciprocal.

Softmax normalization runs almost entirely on the PE: scores (block-diag K
per head pair) -> exp (+alibi bias as per-partition bias, ScalarE) ->
per-head denominators accumulated into ONE [16,tok] PSUM via per-pair masked
ones matmuls -> one copy + fast reciprocal -> reciprocal broadcast to 128
partitions with a tiny per-pair selector matmul, applied to the unnormalized
AV output with one DVE mul per pair.

Pipelined schedule (the PE HAM halves the clock during sparse phases, so
every latency chain must hide under independent matmul work):
  S1: dn(A), qT(B), av(A), scores(B), outproj(A), norm2(A), dn(B), av(B)
  S2: outproj(B), norm2(B), FFN1 m-loop (both chunks; first 4 m's chunk-B
      groups deferred until norm2(B)'s chain is covered)
  S3: FFN2(A) + woven h_next sq + finish(A)   [A contains the keys]
  S4: FFN2(B) interleaved with the NEXT layer's qT(A'), make_kv', scores(A')
      (or the head's chunk-A groups on the last layer) + finish(B)
qT/head PSUM->SBUF copies live on DVE so ScalarE is free for exp chains;
the exp chains always land before the next dn because gelus/exps alternate
at S2/S4 boundaries.  FFN2 weights are streamed twice (S3/S4 chunk-outer
order) — +8MB DMA, well under the DMA roofline.  Startup: first 128-col x
piece + ssq `ones` land first; first-layer norm runs in 128-col pieces with
sq on DVE (gpsimd's serial 0.6us/slab would gate the chain).
Keep gpsimd lightly loaded: heavy co-activity down-clocks the PE ~20%.
"""

import numpy as np
import ml_dtypes

import concourse.bass as bass
import concourse.mybir as mybir
import concourse.tile as tile
from concourse import bacc
from concourse.bass_utils import run_bass_kernel_spmd

F32 = mybir.dt.float32
BF16 = mybir.dt.bfloat16
AF = mybir.ActivationFunctionType
ALU = mybir.AluOpType
BF = ml_dtypes.bfloat16

B, T, L = 4, 1024, 2
D, TOKD, POSD = 1024, 512, 512
H, HD, KVH, FFN = 16, 64, 4, 4096
INNER, KVI, REP = 1024, 256, 4
EPS = 1e-5

NKEY = 64           # keys that can carry softmax mass (last nonzero: 44)
NTOK = 576          # layer-0 slots per core (64 keys + 512 outputs)
NOUT = 512          # output slots per core
KOFF = 0            # key slots [0, NKEY)
YOFF = 64           # output slots [YOFF, YOFF+NOUT)
CHUNKS0 = [(0, 256), (256, 320)]     # layer 0: all 576 slots
# layer-1 chunk A must sit inside layer-0's chunk-A norm (cols 0:256), since
# the next layer's Q matmuls are emitted inside layer-0's FFN2(B) phase
CHUNKS1 = [(64, 192), (256, 320)]    # layer 1 / head: slots 64:576
CHMAX = 320
NCORES = 8


# ----------------------------------------------------------------------------
# host-side weight preparation
# ----------------------------------------------------------------------------

def _prep_weights(inputs):
    """Fold norms/rotation/scale into weights; emit SBUF-image numpy arrays."""
    qW = np.asarray(inputs["qW"], np.float32)
    vW = np.asarray(inputs["vW"], np.float32)
    oW = np.asarray(inputs["oW"], np.float32)
    ln1 = np.asarray(inputs["ln1_w"], np.float32)
    ln2 = np.asarray(inputs["ln2_w"], np.float32)
    lnf = np.asarray(inputs["lnf_w"], np.float32)
    fc1 = np.asarray(inputs["fc1_W"], np.float32)
    fc2 = np.asarray(inputs["fc2_W"], np.float32)
    fc1_b = np.asarray(inputs["fc1_b"], np.float32)
    fc2_b = np.asarray(inputs["fc2_b"], np.float32)
    headW = np.asarray(inputs["head_W"], np.float32)
    ang = np.asarray(inputs["q_phase_angle"], np.float32)
    slopes = np.exp(np.asarray(inputs["alibi_log_slopes"], np.float32))

    out = {}
    qW_l, kW_l, vW_l, oW_l, f1_l, f2_l = [], [], [], [], [], []
    for l in range(L):
        ln1_tok, ln1_pos = ln1[l, :TOKD], ln1[l, TOKD:]
        qW_e = qW[l] * ln1_pos[:, None]          # [512, 1024] folded ln1
        # K uses the UNrotated, UNscaled first KVI columns
        kW_e = qW_e[:, :KVI].copy()              # [512, 256]
        # rotate q per head then fold 1/sqrt(HD)
        qr = qW_e.reshape(POSD, H, HD // 2, 2)
        c = np.cos(ang[l])[None, :, None]
        s = np.sin(ang[l])[None, :, None]
        e, o = qr[..., 0].copy(), qr[..., 1].copy()
        qr[..., 0] = c * e - s * o
        qr[..., 1] = s * e + c * o
        qW_e = qr.reshape(POSD, INNER) * np.float32(1.0 / np.sqrt(HD))
        vW_e = vW[l] * ln1_tok[:, None]          # [512, 256]
        f1_e = fc1[l] * ln2[l][:, None]          # [1024, 4096]

        # SBUF images (lhsT layout: [partition=k%128, kslab, mcols])
        qW_l.append(qW_e.reshape(4, 128, INNER).transpose(1, 0, 2))
        # kW duplicated per kv-head so each q-head can matmul at its own
        # partition base: [128, ks, g, 128] with cols 0:64==64:128==head g
        kw = np.empty((POSD, KVH, 128), np.float32)
        for g in range(KVH):
            blk = kW_e[:, g * HD:(g + 1) * HD]
            kw[:, g, :HD] = blk
            kw[:, g, HD:] = blk
        kW_l.append(kw.reshape(4, 128, KVH, 128).transpose(1, 0, 2, 3))
        vW_l.append(vW_e.reshape(4, 128, KVI).transpose(1, 0, 2))
        oW_l.append(oW[l].reshape(8, 128, D).transpose(1, 0, 2))
        f1_l.append(f1_e.reshape(8, 128, 32, 128).transpose(2, 1, 0, 3))
        f2_l.append(fc2[l].reshape(32, 128, 8, 128).transpose(2, 1, 0, 3))

    out["qW"] = np.ascontiguousarray(np.stack(qW_l)).astype(BF)
    out["kW"] = np.ascontiguousarray(np.stack(kW_l)).astype(BF)
    out["vW"] = np.ascontiguousarray(np.stack(vW_l)).astype(BF)
    out["oW"] = np.ascontiguousarray(np.stack(oW_l)).astype(BF)
    out["f1"] = np.ascontiguousarray(np.stack(f1_l)).astype(BF)
    out["f2"] = np.ascontiguousarray(np.stack(f2_l)).astype(BF)
    hW_e = headW * lnf[:, None]
    out["hW"] = np.ascontiguousarray(
        hW_e.reshape(8, 128, TOKD).transpose(1, 0, 2)).astype(BF)

    # exp bias: -slope * key_index, per partition (keys of the head pair)
    kb = np.empty((128, L, H // 2), np.float32)
    jj = np.arange(64, dtype=np.float32)
    for l in range(L):
        for pr in range(H // 2):
            kb[0:64, l, pr] = -slopes[l, 2 * pr] * jj
            kb[64:128, l, pr] = -slopes[l, 2 * pr + 1] * jj
    out["kb"] = kb
    fb1 = np.zeros((128, L, 32), np.float32)
    fb2 = np.zeros((128, L, 8), np.float32)
    for l in range(L):
        fb1[:, l, :] = fc1_b[l].reshape(32, 128).T
        fb2[:, l, :] = fc2_b[l].reshape(8, 128).T
    # f32 consts packed into one DMA: kb | fb1 | fb2 | eps
    cpf = np.concatenate([kb.reshape(128, 16), fb1.reshape(128, 64),
                          fb2.reshape(128, 16),
                          np.full((128, 1), EPS, np.float32)], axis=1)
    out["cpf"] = np.ascontiguousarray(cpf)
    # per-pair denominator reduction lhsT: [128, pr, 16]; pair pr sums its
    # two heads' key rows into output partitions 2pr (head A) / 2pr+1 (head B)
    dn16 = np.zeros((128, 8, 16), np.float32)
    for pr in range(8):
        dn16[0:64, pr, 2 * pr] = 1.0
        dn16[64:128, pr, 2 * pr + 1] = 1.0
    # bf16 consts packed (per-core cmA appended in _make_in_maps):
    # ones | dn16 | cmA
    out["cpb_shared"] = np.concatenate(
        [np.ones((128, 128), BF), dn16.reshape(128, 128).astype(BF)], axis=1)
    # reciprocal broadcast lhsT per pair: [16, pr, 128]; output row c picks
    # r16 row 2pr (c<64) or 2pr+1 (c>=64)
    selb = np.zeros((16, 8, 128), np.float32)
    for pr in range(8):
        selb[2 * pr, pr, 0:64] = 1.0
        selb[2 * pr + 1, pr, 64:128] = 1.0
    out["selb"] = selb.astype(BF)
    return out


def _core_token_slices(core):
    """Global token rows for this core's 576-row local tensor:
    the 64 key tokens FIRST, then 512 output tokens."""
    b = core // 2
    if core % 2 == 0:
        return b, [(0, 64), (0, 512)]
    return b, [(0, 64), (512, 1024)]


def _make_xt(x, core):
    b, sls = _core_token_slices(core)
    rows = np.concatenate([x[b, a:c] for a, c in sls], axis=0)  # [576, 1024]
    assert rows.shape == (NTOK, D)
    xt = rows.T.reshape(8, 128, NTOK).transpose(1, 0, 2)        # [128, 8, 576]
    return np.ascontiguousarray(xt, dtype=np.float32)


def _make_cmA(core):
    """Mask for chunk-A's first 128 cols.  Cols 0:64 are the key tokens as
    queries (uniform causal).  Cols 64:128 are output tokens 0:64 (even
    cores: causal) or 512:576 (odd cores: all keys visible)."""
    j = np.arange(NKEY)
    causal = (j[:, None] <= j[None, :]).astype(BF)   # keep key j <= query i
    blk2 = causal if core % 2 == 0 else np.ones((NKEY, NKEY), BF)
    half = np.concatenate([causal, blk2], axis=1)    # [64, 128]
    return np.ascontiguousarray(np.concatenate([half, half], axis=0))


def _make_cpb(w, core):
    return np.ascontiguousarray(
        np.concatenate([w["cpb_shared"], _make_cmA(core)], axis=1))


# ----------------------------------------------------------------------------
# device kernel
# ----------------------------------------------------------------------------

_NC_CACHE = {}


def _build_nc():
    if "nc" in _NC_CACHE:
        return _NC_CACHE["nc"]
    nc = bacc.Bacc("TRN2", target_bir_lowering=False, debug=False,
                   num_devices=NCORES)

    xT_d = nc.dram_tensor("xT", [128, 8, NTOK], F32, kind="ExternalInput")
    qW_d = nc.dram_tensor("qW", [L, 128, 4, INNER], BF16, kind="ExternalInput")
    kW_d = nc.dram_tensor("kW", [L, 128, 4, KVH, 128], BF16, kind="ExternalInput")
    vW_d = nc.dram_tensor("vW", [L, 128, 4, KVI], BF16, kind="ExternalInput")
    oW_d = nc.dram_tensor("oW", [L, 128, 8, D], BF16, kind="ExternalInput")
    f1_d = nc.dram_tensor("f1", [L, 32, 128, 8, 128], BF16, kind="ExternalInput")
    f2_d = nc.dram_tensor("f2", [L, 8, 128, 32, 128], BF16, kind="ExternalInput")
    hW_d = nc.dram_tensor("hW", [128, 8, TOKD], BF16, kind="ExternalInput")
    cpf_d = nc.dram_tensor("cpf", [128, 97], F32, kind="ExternalInput")
    cpb_d = nc.dram_tensor("cpb", [128, 384], BF16, kind="ExternalInput")
    selb_d = nc.dram_tensor("selb", [16, 8, 128], BF16, kind="ExternalInput")
    y_d = nc.dram_tensor("y", [128, 4, NOUT], F32, kind="ExternalOutput")

    with tile.TileContext(nc) as tc:
        with (
            tc.tile_pool(name="const", bufs=1) as const,
            tc.tile_pool(name="persist", bufs=1) as persist,
            tc.tile_pool(name="act", bufs=1) as act,
            tc.tile_pool(name="wpool", bufs=1) as wpool,
            tc.tile_pool(name="wstream", bufs=4) as wstream,
            tc.tile_pool(name="small", bufs=2) as small,
            tc.tile_pool(name="attn", bufs=1) as attn,
            tc.tile_pool(name="ps", bufs=8, space="PSUM") as ps,
        ):
            # Startup DMAs, ordered for the first-norm critical path: the
            # first 128-col x piece (contains the keys) and the ssq `ones`
            # land first, then qW/kW/vW for Q + make_kv, then the rest.
            xT = persist.tile([128, 8, NTOK], F32)
            cpf_t = const.tile([128, 97], F32)
            cpb_t = const.tile([128, 384], BF16)
            selb_t = const.tile([16, 8, 128], BF16)
            nc.sync.dma_start(xT[:, :, 0:128], xT_d.ap()[:, :, 0:128])
            nc.sync.dma_start(cpb_t[:], cpb_d.ap())
            nc.sync.dma_start(cpf_t[:], cpf_d.ap())
            nc.sync.dma_start(xT[:, :, 128:256], xT_d.ap()[:, :, 128:256])
            nc.sync.dma_start(selb_t[:], selb_d.ap())
            kb_t = cpf_t[:, 0:16].rearrange("p (l h) -> p l h", l=L)
            fb1_t = cpf_t[:, 16:80].rearrange("p (l m) -> p l m", l=L)
            fb2_t = cpf_t[:, 80:96].rearrange("p (l m) -> p l m", l=L)
            eps_t = cpf_t[:, 96:97]
            ones_t = cpb_t[:, 0:128]
            dn16_t = cpb_t[:, 128:256].rearrange("p (r c) -> p r c", r=8)
            cmA_t = cpb_t[:, 256:384]

            def load_weights(l):
                qW_t = wpool.tile([128, 4, INNER], BF16, tag="qw", name="qW_t")
                nc.sync.dma_start(qW_t[:], qW_d.ap()[l])
                kW_t = wpool.tile([128, 4, KVH, 128], BF16, tag="kw",
                                  name="kW_t")
                nc.sync.dma_start(kW_t[:], kW_d.ap()[l])
                vW_t = wpool.tile([128, 4, KVI], BF16, tag="vw", name="vW_t")
                nc.sync.dma_start(vW_t[:], vW_d.ap()[l])
                if l == 0:
                    nc.sync.dma_start(xT[:, :, 256:576],
                                      xT_d.ap()[:, :, 256:576])
                oW_t = wpool.tile([128, 8, D], BF16, tag="ow", name="oW_t")
                nc.sync.dma_start(oW_t[:], oW_d.ap()[l])
                return qW_t, kW_t, vW_t, oW_t

            w0 = load_weights(0)

            # block-diagonal K^T and V per kv-group: [[M_g, 0], [0, M_g]].
            kT2 = persist.tile([128, KVH, 128], BF16)
            v2 = persist.tile([128, KVH, 128], BF16)
            nc.vector.memset(kT2[:], 0.0)
            nc.vector.memset(v2[:], 0.0)

            def norm_sq(sq_t, c0, cn, s, eng=None):
                eng = eng or nc.gpsimd
                eng.tensor_mul(sq_t[:, s, c0:c0 + cn],
                               xT[:, s, c0:c0 + cn],
                               xT[:, s, c0:c0 + cn])

            def norm_finish(out_bf, sq_t, c0, cn):
                """out_bf[:, :, c0:c0+cn] = rmsnorm(xT) (ln weight folded).
                pos-half slabs (4..7) first so Q matmuls can start early."""
                ssq = ps.tile([128, 512], F32, tag="ps")
                for s in range(8):
                    nc.tensor.matmul(ssq[:, :cn], lhsT=ones_t[:],
                                     rhs=sq_t[:, s, c0:c0 + cn],
                                     start=(s == 0), stop=(s == 7))
                sr = small.tile([128, CHMAX], F32, tag="sr", bufs=3)
                nc.scalar.activation(sr[:, :cn], ssq[:, :cn],
                                     AF.Sqrt, bias=eps_t[:, 0:1], scale=1.0 / D)
                nc.vector.reciprocal_approx_fast(sr[:, :cn], sr[:, :cn])
                sr_b4 = sr[:, :cn].unsqueeze(1).broadcast_to([128, 4, cn])
                nc.vector.tensor_mul(out_bf[:, 4:8, c0:c0 + cn],
                                     xT[:, 4:8, c0:c0 + cn], sr_b4)
                nc.vector.tensor_mul(out_bf[:, 0:4, c0:c0 + cn],
                                     xT[:, 0:4, c0:c0 + cn], sr_b4)

            def norm_chunk(out_bf, sq_t, c0, cn, eng=None, split=False):
                for s in range(8):
                    e = eng
                    if split:  # pos-half on DVE (feeds Q first), rest gpsimd
                        e = nc.vector if s >= 4 else nc.gpsimd
                    norm_sq(sq_t, c0, cn, s, eng=e)
                norm_finish(out_bf, sq_t, c0, cn)

            hT0 = act.tile([128, 8, NTOK], BF16, tag="hT", name="hT0")
            sq1 = act.tile([128, 8, NTOK], BF16, tag="sq", name="sq1")
            # First two 128-col pieces of the layer-0 norm: sq on DVE (idle
            # at startup; gpsimd's serial 0.6us/slab would gate the chain).
            norm_chunk(hT0, sq1, 0, 128, eng=nc.vector)
            norm_chunk(hT0, sq1, 128, 128, eng=nc.vector)

            hW_t = const.tile([128, 8, TOKD], BF16)

            # ------------------------------------------------------------
            # per-layer helpers, parameterized by a small state dict
            # ------------------------------------------------------------

            def new_state(l, hT, wts):
                return {
                    "l": l, "hT": hT,
                    "qW": wts[0], "kW": wts[1], "vW": wts[2], "oW": wts[3],
                    "qT": act.tile([128, 8, NTOK], BF16, tag="qT",
                                   name=f"qT{l}"),
                    "oT": act.tile([128, 8, NTOK], BF16, tag="oT",
                                   name=f"oT{l}"),
                    "exps": {}, "r16": {},
                }

            def make_qT(st, c0, cn):
                for ms in range(8):
                    q_ps = ps.tile([128, 512], F32, tag="ps")
                    for s in range(4):
                        nc.tensor.matmul(
                            q_ps[:, :cn],
                            lhsT=st["qW"][:, s, ms * 128:(ms + 1) * 128],
                            rhs=st["hT"][:, 4 + s, c0:c0 + cn],
                            start=(s == 0), stop=(s == 3))
                    nc.vector.tensor_copy(st["qT"][:, ms, c0:c0 + cn],
                                          q_ps[:, :cn])

            def make_kv(st):
                hT = st["hT"]
                # V: keys (slots 0:64), replicated on both partition halves
                v_ps = ps.tile([128, 512], F32, tag="ps")
                for part in (0, 64):
                    for s in range(4):
                        nc.tensor.matmul(v_ps[part:part + 64, :KVI],
                                         lhsT=hT[:, s, 0:NKEY],
                                         rhs=st["vW"][:, s, :],
                                         start=(s == 0), stop=(s == 3))
                for g in range(KVH):
                    nc.vector.tensor_copy(v2[0:64, g, 0:64],
                                          v_ps[0:64, g * HD:(g + 1) * HD])
                    nc.vector.tensor_copy(v2[64:128, g, 64:128],
                                          v_ps[64:128, g * HD:(g + 1) * HD])
                # K^T diagonal blocks
                for g in range(KVH):
                    k_ps = ps.tile([128, 512], F32, tag="ps")
                    for s in range(4):
                        nc.tensor.matmul(k_ps[:, :NKEY],
                                         lhsT=st["kW"][:, s, g, :],
                                         rhs=hT[:, 4 + s, 0:NKEY],
                                         start=(s == 0), stop=(s == 3))
                    nc.vector.tensor_copy(kT2[0:64, g, 0:64],
                                          k_ps[0:64, :NKEY])
                    nc.vector.tensor_copy(kT2[64:128, g, 64:128],
                                          k_ps[64:128, :NKEY])

            def attn_scores(st, ch_idx, c0, cn):
                """exp(scores+alibi) for all pairs (scores matmul + exp on
                ScalarE + causal-mask muls on gpsimd)."""
                l = st["l"]
                exps = []
                for g in range(KVH):
                    for pr in (2 * g, 2 * g + 1):
                        s_ps = ps.tile([128, 512], F32, tag="ps")
                        nc.tensor.matmul(s_ps[:, :cn], lhsT=kT2[:, g, :],
                                         rhs=st["qT"][:, pr, c0:c0 + cn],
                                         start=True, stop=True)
                        e1 = attn.tile([128, CHMAX], BF16, tag="e1",
                                       bufs=12, name="e1")
                        nc.scalar.activation(e1[:, :cn], s_ps[:, :cn],
                                             AF.Exp,
                                             bias=kb_t[:, l, pr:pr + 1])
                        if ch_idx == 0:
                            if l == 0:
                                # cols 0:64 key-queries (uniform causal),
                                # cols 64:128 per-core
                                nc.gpsimd.tensor_mul(e1[:, 0:128],
                                                     e1[:, 0:128], cmA_t[:])
                            else:
                                # chunk starts at slot 64: only cols 0:64
                                # (output tokens 0:64 / 512:576) need a mask
                                nc.gpsimd.tensor_mul(e1[:, 0:NKEY],
                                                     e1[:, 0:NKEY],
                                                     cmA_t[:, 64:128])
                        exps.append(e1)
                st["exps"][ch_idx] = exps

            def attn_dn(st, ch_idx, c0, cn):
                """Per-head denominators -> one [16,cn] PSUM -> 1/d -> r16.
                Emitted after independent matmul work so the PE isn't parked
                behind the exp chain."""
                exps = st["exps"][ch_idx]
                dn_ps = ps.tile([128, 512], F32, tag="ps")
                for pr in range(8):
                    nc.tensor.matmul(dn_ps[0:16, :cn],
                                     lhsT=dn16_t[:, pr, :],
                                     rhs=exps[pr][:, :cn],
                                     start=(pr == 0), stop=(pr == 7))
                dnsb = attn.tile([16, CHMAX], F32, tag="dnsb", bufs=2)
                nc.vector.tensor_copy(dnsb[:, :cn], dn_ps[0:16, :cn])
                nc.vector.reciprocal_approx_fast(dnsb[:, :cn], dnsb[:, :cn])
                r16 = attn.tile([16, CHMAX], BF16, tag="r16", bufs=2)
                nc.vector.tensor_copy(r16[:, :cn], dnsb[:, :cn])
                st["r16"][ch_idx] = r16

            def attn_av(st, ch_idx, c0, cn):
                """AV (unnormalized), broadcast 1/denom via rank-2 matmul,
                normalize into oT with one DVE mul per pair."""
                r16 = st["r16"][ch_idx]
                exps = st["exps"][ch_idx]
                for g in range(KVH):
                    for pr in (2 * g, 2 * g + 1):
                        av_ps = ps.tile([128, 512], F32, tag="ps")
                        nc.tensor.matmul(av_ps[:, :cn], lhsT=v2[:, g, :],
                                         rhs=exps[pr][:, :cn],
                                         start=True, stop=True)
                        rb_ps = ps.tile([128, 512], F32, tag="ps")
                        nc.tensor.matmul(rb_ps[:, :cn],
                                         lhsT=selb_t[:, pr, :],
                                         rhs=r16[0:16, :cn],
                                         start=True, stop=True)
                        rb_sb = attn.tile([128, CHMAX], BF16, tag="rb",
                                          bufs=3, name="rb_sb")
                        nc.vector.tensor_copy(rb_sb[:, :cn], rb_ps[:, :cn])
                        nc.vector.tensor_mul(st["oT"][:, pr, c0:c0 + cn],
                                             av_ps[:, :cn], rb_sb[:, :cn])

            def outproj(st, c0, cn):
                for ms in range(8):
                    o_ps = ps.tile([128, 512], F32, tag="ps")
                    for ks in range(8):
                        nc.tensor.matmul(
                            o_ps[:, :cn],
                            lhsT=st["oW"][:, ks, ms * 128:(ms + 1) * 128],
                            rhs=st["oT"][:, ks, c0:c0 + cn],
                            start=(ks == 0), stop=(ks == 7))
                    nc.vector.tensor_add(xT[:, ms, c0:c0 + cn],
                                         o_ps[:, :cn],
                                         xT[:, ms, c0:c0 + cn])

            def head_group(hf, m, c0, cn):
                yst = small.tile([128, CHMAX], F32, tag="yst", bufs=2)
                y_ps = ps.tile([128, 512], F32, tag="ps")
                # contract pos-half slabs first: the final norm finishes
                # them first, so the head can start earlier
                for ks in (4, 5, 6, 7, 0, 1, 2, 3):
                    nc.tensor.matmul(y_ps[:, :cn],
                                     lhsT=hW_t[:, ks, m * 128:(m + 1) * 128],
                                     rhs=hf[:, ks, c0:c0 + cn],
                                     start=(ks == 4), stop=(ks == 3))
                nc.vector.tensor_copy(yst[:, :cn], y_ps[:, :cn])
                nc.sync.dma_start(y_d.ap()[:, m, c0 - YOFF:c0 - YOFF + cn],
                                  yst[:, :cn])

            # ------------------------------------------------------------
            # layer 0 prologue
            # ------------------------------------------------------------
            st = new_state(0, hT0, w0)
            make_qT(st, 0, 128)
            make_kv(st)
            make_qT(st, 128, 128)
            attn_scores(st, 0, *CHUNKS0[0])
            # ln1 for chunk B: pos-half sq on DVE (feeds qT(B)), rest gpsimd
            norm_chunk(hT0, sq1, 256, 320, split=True)

            for l in range(L):
                A, Bc = (CHUNKS0 if l == 0 else CHUNKS1)
                hT = st["hT"]

                if l == L - 1:
                    nc.sync.dma_start(hW_t[:], hW_d.ap())

                # ---- S1: attention back-half, chains hidden under matmuls
                h2 = act.tile([128, 8, NTOK], BF16, tag="hT2", name=f"h2_{l}")
                sq2 = act.tile([128, 8, NTOK], BF16, tag="sq", name=f"sq2_{l}")
                if l == 0:
                    make_qT(st, *Bc)      # ln1(B) lands before exps finish
                    attn_dn(st, 0, *A)
                    attn_av(st, 0, *A)
                    attn_scores(st, 1, *Bc)
                    outproj(st, *A)
                    norm_chunk(h2, sq2, *A)
                else:
                    # dn/av(A) were woven into the previous layer's FFN2(B),
                    # so outproj(A) is ready the moment FFN2 ends and the
                    # prev finish(B) chain hides under it
                    outproj(st, *A)
                    make_qT(st, *Bc)
                    norm_chunk(h2, sq2, *A)
                    attn_scores(st, 1, *Bc)

                h_next = act.tile([128, 8, NTOK], BF16, tag="hT",
                                  name=f"h_next{l}")
                sq_next = act.tile([128, 8, NTOK], BF16, tag="sq",
                                   name=f"sq_next{l}")
                gT = act.tile([128, 32, NTOK], BF16, tag="gT", name=f"gT{l}")

                def f1_group(m, f1w, c0, cn):
                    f_ps = ps.tile([128, 512], F32, tag="ps")
                    for ks in range(8):
                        nc.tensor.matmul(f_ps[:, :cn], lhsT=f1w[:, ks, :],
                                         rhs=h2[:, ks, c0:c0 + cn],
                                         start=(ks == 0), stop=(ks == 7))
                    nc.scalar.activation(gT[:, m, c0:c0 + cn], f_ps[:, :cn],
                                         AF.Gelu, bias=fb1_t[:, l, m:m + 1])

                def load_f1w(m):
                    f1w = wstream.tile([128, 8, 128], BF16, tag="f1w",
                                       bufs=8)
                    nc.sync.dma_start(f1w[:], f1_d.ap()[l, m])
                    return f1w

                # ---- S2: FFN1 chunk-A prefix covers the exp(B) chain, then
                # the attention-B back-half, then the main FFN1 loop.  The
                # first 7 m's chunk-B groups are deferred until norm2(B)'s
                # finish chain has completed.
                pend_f1 = []
                for m in range(3):
                    f1w = load_f1w(m)
                    f1_group(m, f1w, *A)
                    pend_f1.append((m, f1w))
                attn_dn(st, 1, *Bc)
                attn_av(st, 1, *Bc)
                outproj(st, *Bc)
                norm_chunk(h2, sq2, *Bc)
                for m in range(3, 32):
                    f1w = load_f1w(m)
                    f1_group(m, f1w, *A)
                    if m < 7:
                        pend_f1.append((m, f1w))
                    else:
                        f1_group(m, f1w, *Bc)
                    if m == 6:
                        for mm, fw in pend_f1:
                            f1_group(mm, fw, *Bc)
                if l + 1 < L:
                    w_next = load_weights(l + 1)

                def ffn2_group(ms, f2w_h, c0, cn):
                    f_ps = ps.tile([128, 512], F32, tag="ps")
                    for ks in range(32):
                        nc.tensor.matmul(f_ps[:, :cn],
                                         lhsT=f2w_h[ks // 16][:, ks % 16, :],
                                         rhs=gT[:, ks, c0:c0 + cn],
                                         start=(ks == 0), stop=(ks == 31))
                    nc.vector.scalar_tensor_tensor(
                        xT[:, ms, c0:c0 + cn], f_ps[:, :cn],
                        fb2_t[:, l, ms:ms + 1], xT[:, ms, c0:c0 + cn],
                        op0=ALU.add, op1=ALU.add)

                def load_f2w(ms):
                    f2w_a = wstream.tile([128, 16, 128], BF16, tag="f2w",
                                         bufs=6)
                    nc.sync.dma_start(f2w_a[:], f2_d.ap()[l, ms][:, 0:16, :])
                    f2w_b = wstream.tile([128, 16, 128], BF16, tag="f2w",
                                         bufs=6)
                    nc.sync.dma_start(f2w_b[:], f2_d.ap()[l, ms][:, 16:32, :])
                    return [f2w_a, f2w_b]

                # ---- S3: FFN2(A) + woven h_next sq + finish(A)
                # (A contains the keys -> next layer's K/V input is ready)
                for ms in range(8):
                    ffn2_group(ms, load_f2w(ms), *A)
                    norm_sq(sq_next, A[0], A[1], ms)
                norm_finish(h_next, sq_next, *A)

                # ---- S4: FFN2(B) interleaved with next layer's front-end
                # (or the head's chunk-A groups on the last layer)
                if l + 1 < L:
                    st_next = new_state(l + 1, h_next, w_next)
                nextA = CHUNKS1[0]
                for ms in range(8):
                    ffn2_group(ms, load_f2w(ms), *Bc)
                    norm_sq(sq_next, Bc[0], Bc[1], ms)
                    if l + 1 < L:
                        # the whole next-layer attention front-end (through
                        # av) hides inside FFN2(B): S1 can then open with
                        # outproj, keeping the PE dense across the boundary
                        if ms == 1:
                            make_qT(st_next, *nextA)
                        elif ms == 2:
                            make_kv(st_next)
                        elif ms == 3:
                            attn_scores(st_next, 0, *nextA)
                        elif ms == 5:
                            attn_dn(st_next, 0, *nextA)
                        elif ms == 6:
                            attn_av(st_next, 0, *nextA)
                    else:
                        if 2 <= ms <= 5:
                            head_group(h_next, ms - 2, *CHUNKS1[0])
                norm_finish(h_next, sq_next, *Bc)

                if l + 1 < L:
                    st = st_next

            # ---- tail: head chunk B
            for m in range(4):
                head_group(h_next, m, *CHUNKS1[1])

    nc.compile()
    _NC_CACHE["nc"] = nc
    return nc


# ----------------------------------------------------------------------------
# entry point
# ----------------------------------------------------------------------------

WKEYS = ("qW", "kW", "vW", "oW", "f1", "f2", "hW", "cpf", "selb")


def _make_in_maps(inputs):
    x = np.asarray(inputs["x"], np.float32)
    w = _prep_weights(inputs)
    in_maps = []
    for core in range(NCORES):
        m = {k: w[k] for k in WKEYS}
        m["xT"] = _make_xt(x, core)
        m["cpb"] = _make_cpb(w, core)
        in_maps.append(m)
    return in_maps


def kernel(**inputs) -> np.ndarray:
    nc = _build_nc()
    in_maps = _make_in_maps(inputs)

    res = run_bass_kernel_spmd(nc, in_maps, core_ids=list(range(NCORES)))
    out = np.empty((B, T, TOKD), np.float32)
    for core in range(NCORES):
        yb = np.asarray(res.results[core]["y"])          # [128, 4, 512]
        yl = yb.transpose(2, 1, 0).reshape(NOUT, TOKD)   # [512, 512]
        b = core // 2
        if core % 2 == 0:
            out[b, 0:512] = yl
        else:
            out[b, 512:1024] = yl
    return out
